# revision 1
# baseline (speedup 1.0000x reference)
"""KD feature-level smooth-L1 loss kernel for Trainium2 (8 NeuronCores).

Math (per batch sample b over (C,H,W) = 256*64*64 = N elements):
  t_norm = (t - mean) * rsqrt(var + eps)          # LayerNorm, no affine
  d   = |t_norm - s|
  kd  = where(d <= 2, d*d/4, d - 1)               # smooth-L1, beta=2
  out = mean_b( sum_chw(kd) )

Device-side decomposition (per sample, rs = 1/sqrt(var+eps), std = 1/rs):
  x  = t - (s*std + mean)        ->  d = rs*|x|
  dt = |x|                       ->  S_d  = sum(dt)     (ACT Abs + accum)
  mt = min(dt, 2*std)            ->  S_m  = sum(mt)     (DVE TS-min + accum)
  S_q = sum(mt^2)                                      (ACT Square + accum)
  sum(kd) = rs*(S_d - S_m) + 0.25*rs^2*S_q
Sharding: pure data parallel, 4 samples per core; host sums 8 partial
outputs and divides by 32.
"""

import os
from contextlib import ExitStack

import numpy as np

import concourse.bass as bass
import concourse.mybir as mybir
import concourse.tile as tile
from concourse import bacc
from concourse.bass_utils import run_bass_kernel_spmd

B, C, H, W = 32, 256, 64, 64
N_CORES = 8
BPC = B // N_CORES            # samples per core
P = 128
N = C * H * W                 # 1048576 elements per sample
FD = N // P                   # 8192 free-dim per partition
NCH = 4                       # loss chunks per sample
CH = FD // NCH                # 2048
EPS = 1e-5
BETA = 2.0
LOSS_WEIGHT = 1.0

f32 = mybir.dt.float32
AF = mybir.ActivationFunctionType
OP = mybir.AluOpType
AX = mybir.AxisListType


def _build_kernel(ctx: ExitStack, tc: "tile.TileContext", out_ap, teacher, stu):
    nc = tc.nc

    const_pool = ctx.enter_context(tc.tile_pool(name="const", bufs=1))
    t_pool = ctx.enter_context(tc.tile_pool(name="t", bufs=2))
    s_pool = ctx.enter_context(tc.tile_pool(name="s", bufs=2))
    v_pool = ctx.enter_context(tc.tile_pool(name="v", bufs=2))
    x_pool = ctx.enter_context(tc.tile_pool(name="x", bufs=2))
    d_pool = ctx.enter_context(tc.tile_pool(name="d", bufs=2))
    m_pool = ctx.enter_context(tc.tile_pool(name="m", bufs=2))
    dead_pool = ctx.enter_context(tc.tile_pool(name="dead", bufs=2))
    sums_pool = ctx.enter_context(tc.tile_pool(name="sums", bufs=2))
    tiny_pool = ctx.enter_context(tc.tile_pool(name="tiny", bufs=2))
    ps_sumt_pool = ctx.enter_context(tc.tile_pool(name="ps_sumt", bufs=2, space="PSUM"))
    ps_misc_pool = ctx.enter_context(tc.tile_pool(name="ps_misc", bufs=2, space="PSUM"))

    ones = const_pool.tile([P, 1], f32)
    nc.vector.memset(ones[:], 1.0)
    staging = const_pool.tile([1, 16 * BPC], f32)
    nc.vector.memset(staging[:], 0.0)

    for b in range(BPC):
        # ---------------- load teacher sample ----------------
        t_sb = t_pool.tile([P, FD], f32)
        nc.sync.dma_start(t_sb[:, 0 : FD // 2], teacher[b, :, 0 : FD // 2])
        nc.sync.dma_start(t_sb[:, FD // 2 : FD], teacher[b, :, FD // 2 : FD])

        # issue student loads early so they overlap the stats phase
        s_tiles = []
        for i in range(2):
            s_sb = s_pool.tile([P, FD // 2], f32)
            nc.sync.dma_start(s_sb[:], stu[b, :, i * (FD // 2) : (i + 1) * (FD // 2)])
            s_tiles.append(s_sb)

        # per-sample per-partition partial sums:
        # cols 0:4  sum|x| per chunk   4:8  sum(mt)   8:12 sum(mt^2)  12:16 sum(t^2)
        sums = sums_pool.tile([P, 16], f32)

        # ---------------- stats: S_t (PE), S_tt (DVE TTR) ----------------
        ps_t = ps_sumt_pool.tile([1, 512], f32)
        nmm = FD // 512
        for k in range(nmm):
            nc.tensor.matmul(
                ps_t[:, :],
                ones[:, :],
                t_sb[:, k * 512 : (k + 1) * 512],
                start=(k == 0),
                stop=(k == nmm - 1),
            )
        for c in range(NCH):
            sl = slice(c * CH, (c + 1) * CH)
            ttdead = dead_pool.tile([P, CH], f32)
            nc.vector.scalar_tensor_tensor(
                ttdead[:],
                t_sb[:, sl],
                1.0,
                t_sb[:, sl],
                op0=OP.mult,
                op1=OP.mult,
                accum_out=sums[:, 12 + c : 13 + c],
            )

        ps_m = ps_misc_pool.tile([1, 16], f32)
        nc.tensor.matmul(ps_m[:, 12:16], ones[:, :], sums[:, 12:16], start=True, stop=True)

        # ---------------- tiny scalar math ----------------
        # bb cols: 0=std 1=mean 2=thr 3..11 scratch
        bb = tiny_pool.tile([1, 16], f32)
        st = bb[0:1, 3:4]
        nc.vector.reduce_sum(out=st, in_=ps_t[:, :], axis=AX.X)
        stt = bb[0:1, 4:5]
        nc.vector.reduce_sum(out=stt, in_=ps_m[0:1, 12:16], axis=AX.X)
        mean = bb[0:1, 1:2]
        nc.vector.tensor_scalar(mean, st, 1.0 / N, None, op0=OP.mult)
        ve_a = bb[0:1, 5:6]
        nc.vector.tensor_scalar(ve_a, stt, 1.0 / N, EPS, op0=OP.mult, op1=OP.add)
        msq = bb[0:1, 6:7]
        nc.vector.tensor_tensor(msq, mean, mean, op=OP.mult)
        ve = bb[0:1, 7:8]
        nc.vector.tensor_tensor(ve, ve_a, msq, op=OP.subtract)
        inv_ve = bb[0:1, 8:9]
        nc.vector.reciprocal(inv_ve, ve)
        rs = bb[0:1, 9:10]
        nc.scalar.activation(rs, inv_ve, AF.Sqrt)  # rs0 ~= 1/sqrt(ve) (table)
        # two Newton iterations: rs <- rs*(1.5 - 0.5*ve*rs^2)
        for it in range(2):
            r2 = bb[0:1, 10:11]
            nc.vector.tensor_tensor(r2, rs, rs, op=OP.mult)
            pv = bb[0:1, 11:12]
            nc.vector.tensor_tensor(pv, r2, ve, op=OP.mult)
            hh = bb[0:1, 12:13]
            nc.vector.tensor_scalar(hh, pv, -0.5, 1.5, op0=OP.mult, op1=OP.add)
            rs_new = bb[0:1, 13 + it : 14 + it]
            nc.vector.tensor_tensor(rs_new, rs, hh, op=OP.mult)
            rs = rs_new
        stdv = bb[0:1, 0:1]
        nc.vector.tensor_tensor(stdv, ve, rs, op=OP.mult)  # std = ve*rs = sqrt(ve)
        thr = bb[0:1, 2:3]
        nc.vector.tensor_scalar(thr, stdv, BETA, None, op0=OP.mult)

        bcast = tiny_pool.tile([P, 3], f32)
        nc.gpsimd.partition_broadcast(bcast[:, 0:3], bb[0:1, 0:3])
        std_vec = bcast[:, 0:1]
        mean_vec = bcast[:, 1:2]
        thr_vec = bcast[:, 2:3]

        # ---------------- loss passes ----------------
        for c in range(NCH):
            tsl = slice(c * CH, (c + 1) * CH)
            ssb = s_tiles[c // 2]
            ssl = slice((c % 2) * CH, (c % 2 + 1) * CH)

            v = v_pool.tile([P, CH], f32)
            nc.scalar.activation(v[:], ssb[:, ssl], AF.Identity, bias=mean_vec, scale=std_vec)
            x = x_pool.tile([P, CH], f32)
            nc.vector.tensor_tensor(x[:], t_sb[:, tsl], v[:], op=OP.subtract)
            d = d_pool.tile([P, CH], f32)
            nc.scalar.activation(d[:], x[:], AF.Abs, accum_out=sums[:, c : c + 1])
            m = m_pool.tile([P, CH], f32)
            nc.vector.tensor_scalar(
                m[:],
                d[:],
                thr_vec,
                0.0,
                op0=OP.min,
                op1=OP.add,
                accum_out=sums[:, 4 + c : 5 + c],
            )
            # dead output written over x (x is dead after Abs)
            nc.scalar.activation(x[:], m[:], AF.Square, accum_out=sums[:, 8 + c : 9 + c])

        # partition-reduce the 12 loss partials in one matmul
        nc.tensor.matmul(ps_m[:, 0:12], ones[:, :], sums[:, 0:12], start=True, stop=True)
        nc.vector.tensor_copy(staging[0:1, 16 * b : 16 * b + 12], ps_m[0:1, 0:12])
        nc.vector.tensor_copy(staging[0:1, 16 * b + 12 : 16 * b + 13], rs)
        nc.vector.tensor_copy(staging[0:1, 16 * b + 13 : 16 * b + 14], stdv)
        nc.vector.tensor_copy(staging[0:1, 16 * b + 14 : 16 * b + 15], bb[0:1, 1:2])
        nc.vector.tensor_copy(staging[0:1, 16 * b + 15 : 16 * b + 16], ve)

    nc.sync.dma_start(out_ap[:, :], staging[:, :])


_CACHED = {}


def _get_nc():
    if "nc" in _CACHED:
        return _CACHED["nc"]
    nc = bacc.Bacc(
        "TRN2",
        target_bir_lowering=False,
        debug=False,
        enable_asserts=False,
        num_devices=N_CORES,
    )
    teacher = nc.dram_tensor("teacher", [BPC, P, FD], f32, kind="ExternalInput").ap()
    stu = nc.dram_tensor("stu", [BPC, P, FD], f32, kind="ExternalInput").ap()
    out = nc.dram_tensor("out", [1, 16 * BPC], f32, kind="ExternalOutput").ap()
    with tile.TileContext(nc) as tc:
        with ExitStack() as ctx:
            _build_kernel(ctx, tc, out, teacher, stu)
    nc.compile()
    _CACHED["nc"] = nc
    return nc


def _combine(parts):
    """parts: list of 8 arrays [1, 16*BPC] -> scalar loss (float64 math)."""
    losses = []
    for r in parts:
        r = np.asarray(r, dtype=np.float64).reshape(BPC, 16)
        S_d = r[:, 0:4].sum(axis=1)
        S_m = r[:, 4:8].sum(axis=1)
        S_q = r[:, 8:12].sum(axis=1)
        rs = r[:, 12]
        losses.append(rs * (S_d - S_m) + 0.25 * rs * rs * S_q)
    losses = np.concatenate(losses)
    return np.float32(LOSS_WEIGHT * losses.mean())


def run(inputs: dict, trace: bool = False):
    teacher = np.ascontiguousarray(np.asarray(inputs["teacher_feat"], dtype=np.float32))
    stu = np.ascontiguousarray(np.asarray(inputs["stu_feat"], dtype=np.float32))
    assert teacher.shape == (B, C, H, W) and stu.shape == (B, C, H, W)
    tch = teacher.reshape(N_CORES, BPC, P, FD)
    sch = stu.reshape(N_CORES, BPC, P, FD)
    in_maps = [
        {"teacher": np.ascontiguousarray(tch[i]), "stu": np.ascontiguousarray(sch[i])}
        for i in range(N_CORES)
    ]
    nc = _get_nc()
    res = run_bass_kernel_spmd(nc, in_maps, core_ids=list(range(N_CORES)), trace=trace)
    parts = [res.results[i]["out"] for i in range(N_CORES)]
    return _combine(parts), res


def kernel(**inputs) -> np.ndarray:
    out, _ = run(inputs, trace=False)
    return np.asarray(out, dtype=np.float32)


if __name__ == "__main__":
    rng = np.random.default_rng(0)
    ins = {
        "teacher_feat": rng.standard_normal((B, C, H, W), dtype=np.float32),
        "stu_feat": rng.standard_normal((B, C, H, W), dtype=np.float32),
    }
    print(kernel(**ins))



# revision 4
# speedup vs baseline: 1.0397x; 1.0397x over previous
"""KD feature-level smooth-L1 loss kernel for Trainium2 (8 NeuronCores).

Math (per batch sample b over (C,H,W) = 256*64*64 = N elements):
  t_norm = (t - mean) * rsqrt(var + eps)          # LayerNorm, no affine
  d   = |t_norm - s|
  kd  = where(d <= 2, d*d/4, d - 1)               # smooth-L1, beta=2
  out = mean_b( sum_chw(kd) )

v2: work in normalized space with bf16 intermediates.
  x  = t*rs - s                 (DVE STT, bf16 2x)
  d  = |x - mean*rs|            (ACT Abs with bias, accum S_d)
  m  = min(d, 2)                (DVE TS imm, bf16 4x)
  q  = m^2                      (ACT Square, accum S_q)
  S_m via bf16 ones-matmul partition reduction (PE)
  sum(kd) = S_d - S_m + 0.25*S_q
Inputs are cast fp32->bf16 during the (SWDGE) DMA, halving SBUF
footprint and doubling DVE throughput; stats = sum(t), sum(t^2) use a
bf16 STT square pass + bf16 ones-matmuls instead of fp32 matmuls.
Sharding: pure data parallel, 4 samples per core; host combines 8
partial outputs.
"""

import os
from contextlib import ExitStack

import numpy as np

import concourse.bass as bass
import concourse.mybir as mybir
import concourse.tile as tile
from concourse import bacc
from concourse.bass_utils import run_bass_kernel_spmd

B, C, H, W = 32, 256, 64, 64
N_CORES = 8
BPC = B // N_CORES            # samples per core
P = 128
N = C * H * W                 # 1048576 elements per sample
FD = N // P                   # 8192 free-dim per partition
NCH = 2                       # loss chunks per sample
CH = FD // NCH                # 4096
MM = 512                      # matmul free-dim block (PSUM bank width)
EPS = 1e-5
BETA = 2.0
LOSS_WEIGHT = 1.0

f32 = mybir.dt.float32
bf16 = mybir.dt.bfloat16
AF = mybir.ActivationFunctionType
OP = mybir.AluOpType
AX = mybir.AxisListType


def _build_kernel(ctx: ExitStack, tc: "tile.TileContext", out_ap, teacher, stu):
    nc = tc.nc

    const_pool = ctx.enter_context(tc.tile_pool(name="const", bufs=1))
    t_pool = ctx.enter_context(tc.tile_pool(name="t", bufs=1))
    s_pool = ctx.enter_context(tc.tile_pool(name="s", bufs=1))
    sq_pool = ctx.enter_context(tc.tile_pool(name="sq", bufs=2))
    x_pool = ctx.enter_context(tc.tile_pool(name="x", bufs=2))
    d_pool = ctx.enter_context(tc.tile_pool(name="d", bufs=2))
    m_pool = ctx.enter_context(tc.tile_pool(name="m", bufs=2))
    sums_pool = ctx.enter_context(tc.tile_pool(name="sums", bufs=2))
    tiny_pool = ctx.enter_context(tc.tile_pool(name="tiny", bufs=2))
    bc_pool = ctx.enter_context(tc.tile_pool(name="bc", bufs=2))
    ps_t_pool = ctx.enter_context(tc.tile_pool(name="ps_t", bufs=2, space="PSUM"))
    ps_tt_pool = ctx.enter_context(tc.tile_pool(name="ps_tt", bufs=2, space="PSUM"))
    ps_m_pool = ctx.enter_context(tc.tile_pool(name="ps_m", bufs=2, space="PSUM"))
    ps_sm_pool = ctx.enter_context(tc.tile_pool(name="ps_sm", bufs=2, space="PSUM"))

    ones_bf = const_pool.tile([P, 1], bf16)
    nc.vector.memset(ones_bf[:], 1.0)
    ones_f32 = const_pool.tile([P, 1], f32)
    nc.vector.memset(ones_f32[:], 1.0)
    staging = const_pool.tile([1, 8 * BPC], f32)
    nc.vector.memset(staging[:], 0.0)

    # ---------------- all input DMAs up front (SWDGE fp32->bf16 cast) ------
    t_tiles = [t_pool.tile([P, FD], bf16, name=f"t{b}") for b in range(BPC)]
    s_tiles = [s_pool.tile([P, FD], bf16, name=f"s{b}") for b in range(BPC)]
    HF = FD // 2

    def dma_t(b):
        nc.gpsimd.dma_start(t_tiles[b][:, 0:HF], teacher[b, :, 0:HF])
        nc.gpsimd.dma_start(t_tiles[b][:, HF:FD], teacher[b, :, HF:FD])

    def dma_s(b):
        nc.gpsimd.dma_start(s_tiles[b][:, 0:HF], stu[b, :, 0:HF])
        nc.gpsimd.dma_start(s_tiles[b][:, HF:FD], stu[b, :, HF:FD])

    # order: teacher leads student by one sample so stats stay ahead of loss
    dma_t(0)
    dma_t(1)
    dma_s(0)
    dma_t(2)
    dma_s(1)
    dma_t(3)
    dma_s(2)
    dma_s(3)

    state = {}  # per-sample: ps_t, ps_tt, bb, rs_vec, nmrs_vec, rs_f, mean

    def stats(b):
        t_sb = t_tiles[b]
        ps_t = ps_t_pool.tile([1, MM], f32)
        ps_tt = ps_tt_pool.tile([1, MM], f32)
        sqs = []
        for c in range(NCH):
            sl = slice(c * CH, (c + 1) * CH)
            sq = sq_pool.tile([P, CH], bf16)
            nc.vector.scalar_tensor_tensor(
                sq[:], t_sb[:, sl], 1.0, t_sb[:, sl], op0=OP.mult, op1=OP.mult
            )
            sqs.append(sq)
        nmm = FD // MM
        for k in range(nmm):
            nc.tensor.matmul(
                ps_t[:, :],
                ones_bf[:, :],
                t_sb[:, k * MM : (k + 1) * MM],
                start=(k == 0),
                stop=(k == nmm - 1),
            )
        nmm_c = CH // MM
        for c in range(NCH):
            for k in range(nmm_c):
                nc.tensor.matmul(
                    ps_tt[:, :],
                    ones_bf[:, :],
                    sqs[c][:, k * MM : (k + 1) * MM],
                    start=(c == 0 and k == 0),
                    stop=(c == NCH - 1 and k == nmm_c - 1),
                )
        state[b] = {"ps_t": ps_t, "ps_tt": ps_tt}

    def tiny(b):
        st_ = state[b]
        bb = tiny_pool.tile([1, 20], f32)
        st = bb[0:1, 3:4]
        nc.vector.reduce_sum(out=st, in_=st_["ps_t"][:, :], axis=AX.X)
        stt = bb[0:1, 4:5]
        nc.vector.reduce_sum(out=stt, in_=st_["ps_tt"][:, :], axis=AX.X)
        mean = bb[0:1, 2:3]
        nc.vector.tensor_scalar(mean, st, 1.0 / N, None, op0=OP.mult)
        e2 = bb[0:1, 5:6]
        nc.vector.tensor_scalar(e2, stt, 1.0 / N, EPS, op0=OP.mult, op1=OP.add)
        msq = bb[0:1, 6:7]
        nc.vector.tensor_tensor(msq, mean, mean, op=OP.mult)
        ve = bb[0:1, 7:8]
        nc.vector.tensor_tensor(ve, e2, msq, op=OP.subtract)
        inv_ve = bb[0:1, 8:9]
        nc.vector.reciprocal(inv_ve, ve)
        rs = bb[0:1, 9:10]
        nc.scalar.activation(rs, inv_ve, AF.Sqrt)  # rs0 ~= 1/sqrt(ve) (table)
        # two Newton iterations: rs <- rs*(1.5 - 0.5*ve*rs^2)
        for it in range(2):
            r2 = bb[0:1, 10 + 3 * it : 11 + 3 * it]
            nc.vector.tensor_tensor(r2, rs, rs, op=OP.mult)
            pv = bb[0:1, 11 + 3 * it : 12 + 3 * it]
            nc.vector.tensor_tensor(pv, r2, ve, op=OP.mult)
            hh = bb[0:1, 12 + 3 * it : 13 + 3 * it]
            nc.vector.tensor_scalar(hh, pv, -0.5, 1.5, op0=OP.mult, op1=OP.add)
            rs_new = bb[0:1, 0:1] if it == 1 else bb[0:1, 16:17]
            nc.vector.tensor_tensor(rs_new, rs, hh, op=OP.mult)
            rs = rs_new
        # bb col0 = rs (final); col1 = -mean*rs
        mean_rs = bb[0:1, 17:18]
        nc.vector.tensor_tensor(mean_rs, mean, rs, op=OP.mult)
        nc.vector.tensor_scalar(bb[0:1, 1:2], mean_rs, -1.0, None, op0=OP.mult)
        bcast = bc_pool.tile([P, 2], f32)
        nc.gpsimd.partition_broadcast(bcast[:, 0:2], bb[0:1, 0:2])
        st_["bb"] = bb
        st_["rs_f"] = rs
        st_["mean"] = mean
        st_["rs_vec"] = bcast[:, 0:1]
        st_["nmrs_vec"] = bcast[:, 1:2]

    def loss(b):
        st_ = state[b]
        t_sb, s_sb = t_tiles[b], s_tiles[b]
        rs_vec, nmrs_vec = st_["rs_vec"], st_["nmrs_vec"]
        ps_m = ps_m_pool.tile([1, MM], f32)
        sums = sums_pool.tile([P, 8], f32)  # cols 0..1 S_d, 2..3 S_q
        nmm_c = CH // MM
        for c in range(NCH):
            sl = slice(c * CH, (c + 1) * CH)
            x = x_pool.tile([P, CH], bf16)
            nc.vector.scalar_tensor_tensor(
                x[:], t_sb[:, sl], rs_vec, s_sb[:, sl], op0=OP.mult, op1=OP.subtract
            )
            d = d_pool.tile([P, CH], bf16)
            nc.scalar.activation(
                d[:], x[:], AF.Abs, bias=nmrs_vec, accum_out=sums[:, c : c + 1]
            )
            m = m_pool.tile([P, CH], bf16)
            nc.vector.tensor_scalar_min(m[:], d[:], BETA)
            # q written over x (dead after Abs)
            nc.scalar.activation(
                x[:], m[:], AF.Square, accum_out=sums[:, 2 + c : 3 + c]
            )
            for k in range(nmm_c):
                nc.tensor.matmul(
                    ps_m[:, :],
                    ones_bf[:, :],
                    m[:, k * MM : (k + 1) * MM],
                    start=(c == 0 and k == 0),
                    stop=(c == NCH - 1 and k == nmm_c - 1),
                )
        ps_sm = ps_sm_pool.tile([1, 8], f32)
        nc.tensor.matmul(ps_sm[:, 0:4], ones_f32[:, :], sums[:, 0:4], start=True, stop=True)
        o = 8 * b
        nc.vector.tensor_copy(staging[0:1, o : o + 4], ps_sm[0:1, 0:4])
        nc.vector.reduce_sum(out=staging[0:1, o + 4 : o + 5], in_=ps_m[:, :], axis=AX.X)
        nc.vector.tensor_copy(staging[0:1, o + 5 : o + 6], st_["rs_f"])
        nc.vector.tensor_copy(staging[0:1, o + 6 : o + 7], st_["mean"])

    # software pipeline: stats lead loss by one sample
    stats(0)
    tiny(0)
    for b in range(BPC):
        if b + 1 < BPC:
            stats(b + 1)
        loss(b)
        if b + 1 < BPC:
            tiny(b + 1)

    nc.sync.dma_start(out_ap[:, :], staging[:, :])


_CACHED = {}


def _get_nc():
    if "nc" in _CACHED:
        return _CACHED["nc"]
    nc = bacc.Bacc(
        "TRN2",
        target_bir_lowering=False,
        debug=False,
        enable_asserts=False,
        num_devices=N_CORES,
    )
    teacher = nc.dram_tensor("teacher", [BPC, P, FD], f32, kind="ExternalInput").ap()
    stu = nc.dram_tensor("stu", [BPC, P, FD], f32, kind="ExternalInput").ap()
    out = nc.dram_tensor("out", [1, 8 * BPC], f32, kind="ExternalOutput").ap()
    with tile.TileContext(nc) as tc:
        with ExitStack() as ctx:
            _build_kernel(ctx, tc, out, teacher, stu)
    nc.compile()
    _CACHED["nc"] = nc
    return nc


def _combine(parts):
    """parts: list of 8 arrays [1, 8*BPC] -> scalar loss (float64 math)."""
    losses = []
    for r in parts:
        r = np.asarray(r, dtype=np.float64).reshape(BPC, 8)
        S_d = r[:, 0:2].sum(axis=1)
        S_q = r[:, 2:4].sum(axis=1)
        S_m = r[:, 4]
        losses.append(S_d - S_m + 0.25 * S_q)
    losses = np.concatenate(losses)
    return np.float32(LOSS_WEIGHT * losses.mean())


def run(inputs: dict, trace: bool = False):
    teacher = np.ascontiguousarray(np.asarray(inputs["teacher_feat"], dtype=np.float32))
    stu = np.ascontiguousarray(np.asarray(inputs["stu_feat"], dtype=np.float32))
    assert teacher.shape == (B, C, H, W) and stu.shape == (B, C, H, W)
    tch = teacher.reshape(N_CORES, BPC, P, FD)
    sch = stu.reshape(N_CORES, BPC, P, FD)
    in_maps = [
        {"teacher": np.ascontiguousarray(tch[i]), "stu": np.ascontiguousarray(sch[i])}
        for i in range(N_CORES)
    ]
    nc = _get_nc()
    res = run_bass_kernel_spmd(nc, in_maps, core_ids=list(range(N_CORES)), trace=trace)
    parts = [res.results[i]["out"] for i in range(N_CORES)]
    return _combine(parts), res


def kernel(**inputs) -> np.ndarray:
    out, _ = run(inputs, trace=False)
    return np.asarray(out, dtype=np.float32)


if __name__ == "__main__":
    rng = np.random.default_rng(0)
    ins = {
        "teacher_feat": rng.standard_normal((B, C, H, W), dtype=np.float32),
        "stu_feat": rng.standard_normal((B, C, H, W), dtype=np.float32),
    }
    print(kernel(**ins))


# revision 14
# speedup vs baseline: 1.1532x; 1.1092x over previous
"""KD feature-level smooth-L1 loss kernel for Trainium2 (8 NeuronCores).

Math (per batch sample b over (C,H,W) = 256*64*64 = N elements):
  t_norm = (t - mean) * rsqrt(var + eps)          # LayerNorm, no affine
  d   = |t_norm - s|
  kd  = where(d <= 2, d*d/4, d - 1)               # smooth-L1, beta=2
  out = mean_b( sum_chw(kd) )

v4: one fused custom-DVE op computes the whole loss elementwise chain
with an on-op accumulator.  With y = t*rs + (-mean*rs) - s and
c = clamp(y, -2, 2):
  4*kd = y^2 - relu(|y|-2)^2 = c*(2y - c)
so per chunk a single DVE instruction yields per-partition sums of 4*kd.
Stats: sum(t) via bf16 ones-matmuls (PE), sum(t^2) via ACT Square with
free accumulation.  Inputs are cast fp32->bf16 during the SWDGE DMA.
The kernel is then HBM-bound (~94us/core to stream 32 MiB).
Sharding: pure data parallel, 4 samples per core; host combines.
"""

import os
from contextlib import ExitStack
from operator import add as _operator_add

import numpy as np

import concourse.bass as bass
import concourse.mybir as mybir
import concourse.tile as tile
from concourse import bacc
from concourse.bass_utils import run_bass_kernel_spmd

B, C, H, W = 32, 256, 64, 64
N_CORES = 8
BPC = B // N_CORES            # samples per core
P = 128
N = C * H * W                 # 1048576 elements per sample
FD = N // P                   # 8192 free-dim per partition
NCH = 2                       # chunks per sample
CH = FD // NCH                # 4096
MM = 512                      # matmul free-dim block (PSUM bank width)
EPS = 1e-5
BETA = 2.0
LOSS_WEIGHT = 1.0

f32 = mybir.dt.float32
bf16 = mybir.dt.bfloat16
AF = mybir.ActivationFunctionType
OP = mybir.AluOpType
AX = mybir.AxisListType


# ---------------- custom fused DVE op: accum += 4*kd ----------------------
def _register_kd_op():
    import concourse.dve_ops as dve_ops
    from concourse.dve_ops import DveOp
    from concourse.dve_spec import (
        C0,
        C1,
        C2,
        Latch,
        Spec,
        Src0,
        Src1,
        Zero,
        _has_src1,
        lower,
        maxx,
        minn,
    )
    from concourse.dve_table_gen import dve_ver_for
    from concourse.dve_uop import DveOpSpec

    name = "TENSOR_KD_SMOOTHL1"
    if name in dve_ops._SUB_OPCODE_FOR_NAME:
        return next(op for op in dve_ops.OPS if op.name == name)

    # in0 = tn (= t*rs - mean*rs, prescaled on ACT), in1 = s.
    # y = tn - s ; c = clamp(y, -2, 2) ; body = c*(2y - c) = 4*kd
    # 6 ALU ops + 1 accum stage <= 8-stage DVE pipeline.
    y = Src0 - Src1
    c = maxx(minn(y, C2), Latch(Zero - C2))
    body = c * (y + y - c)

    def _ref(in0, in1, c0, c1, c2):
        yv = in0.astype(np.float32) - in1.astype(np.float32)
        cv = np.clip(yv, -c2, c2)
        b = (cv * (2.0 * yv - cv)).astype(np.float32)
        return b, b.reshape(b.shape[0], -1).sum(axis=-1, keepdims=True)

    spec = Spec(body=body, accum=_operator_add, reference=_ref)
    ver = dve_ver_for("TRN2")
    row = max(dve_ops._SUB_OPCODE_FOR_NAME.values()) + 1
    assert row < 0x20
    probe = DveOpSpec(
        name=name, opcode=row, uops=lower(spec, ver=ver), rd1_en=_has_src1(spec)
    )
    op = DveOp(name, spec, subdim=False, uops_sha={ver: probe.sha(ver)})
    dve_ops.OPS.append(op)
    dve_ops.CUSTOM_DVE_SPECS[name] = spec
    dve_ops._SUB_OPCODE_FOR_NAME[name] = row
    return op


KD_OP = _register_kd_op()


def _build_kernel(ctx: ExitStack, tc: "tile.TileContext", out_ap, teacher, stu):
    nc = tc.nc

    const_pool = ctx.enter_context(tc.tile_pool(name="const", bufs=1))
    t_pool = ctx.enter_context(tc.tile_pool(name="t", bufs=1))
    s_pool = ctx.enter_context(tc.tile_pool(name="s", bufs=1))
    dead_pool = ctx.enter_context(tc.tile_pool(name="dead", bufs=2))
    tn_pool = ctx.enter_context(tc.tile_pool(name="tn", bufs=2))
    sums_pool = ctx.enter_context(tc.tile_pool(name="sums", bufs=2))
    tiny_pool = ctx.enter_context(tc.tile_pool(name="tiny", bufs=2))
    bc_pool = ctx.enter_context(tc.tile_pool(name="bc", bufs=2))
    ps_t_pool = ctx.enter_context(tc.tile_pool(name="ps_t", bufs=2, space="PSUM"))
    ps_sm_pool = ctx.enter_context(tc.tile_pool(name="ps_sm", bufs=2, space="PSUM"))

    ones_bf = const_pool.tile([P, 1], bf16)
    nc.vector.memset(ones_bf[:], 1.0)
    ones_f32 = const_pool.tile([P, 1], f32)
    nc.vector.memset(ones_f32[:], 1.0)
    staging = const_pool.tile([1, 8 * BPC], f32)
    nc.vector.memset(staging[:], 0.0)

    # ---------------- all input DMAs up front (SWDGE fp32->bf16 cast) ------
    t_tiles = [t_pool.tile([P, FD], bf16, name=f"t{b}") for b in range(BPC)]
    s_tiles = [s_pool.tile([P, FD], bf16, name=f"s{b}") for b in range(BPC)]
    HF = FD // 2

    def dma_t(b):
        nc.gpsimd.dma_start(t_tiles[b][:, 0:HF], teacher[b, :, 0:HF])
        nc.gpsimd.dma_start(t_tiles[b][:, HF:FD], teacher[b, :, HF:FD])

    def dma_s(b):
        nc.gpsimd.dma_start(s_tiles[b][:, 0:HF], stu[b, :, 0:HF])
        nc.gpsimd.dma_start(s_tiles[b][:, HF:FD], stu[b, :, HF:FD])

    # teacher leads student by one sample so stats stay ahead of loss
    dma_t(0)
    dma_t(1)
    dma_s(0)
    dma_t(2)
    dma_s(1)
    dma_t(3)
    dma_s(2)
    dma_s(3)

    state = {}

    def stats(b):
        t_sb = t_tiles[b]
        ps_t = ps_t_pool.tile([1, MM], f32)
        nmm = FD // MM
        for k in range(nmm):
            nc.tensor.matmul(
                ps_t[:, :],
                ones_bf[:, :],
                t_sb[:, k * MM : (k + 1) * MM],
                start=(k == 0),
                stop=(k == nmm - 1),
            )
        # sums cols: 0:2 = per-chunk sum(t^2) accums, 2:4 = per-chunk sum(4kd)
        sums = sums_pool.tile([P, 8], f32)
        for ci in range(NCH):
            sl = slice(ci * CH, (ci + 1) * CH)
            dead = dead_pool.tile([P, CH], bf16)
            nc.scalar.activation(
                dead[:], t_sb[:, sl], AF.Square, accum_out=sums[:, ci : ci + 1]
            )
        state[b] = {"ps_t": ps_t, "sums": sums}

    def tiny(b):
        st_ = state[b]
        bb = tiny_pool.tile([1, 20], f32)
        ps_sm = ps_sm_pool.tile([1, 8], f32)
        nc.tensor.matmul(
            ps_sm[:, 0:2], ones_f32[:, :], st_["sums"][:, 0:2], start=True, stop=True
        )
        st = bb[0:1, 3:4]
        nc.vector.reduce_sum(out=st, in_=st_["ps_t"][:, :], axis=AX.X)
        stt = bb[0:1, 4:5]
        nc.vector.reduce_sum(out=stt, in_=ps_sm[0:1, 0:2], axis=AX.X)
        mean = bb[0:1, 2:3]
        nc.vector.tensor_scalar(mean, st, 1.0 / N, None, op0=OP.mult)
        e2 = bb[0:1, 5:6]
        nc.vector.tensor_scalar(e2, stt, 1.0 / N, EPS, op0=OP.mult, op1=OP.add)
        msq = bb[0:1, 6:7]
        nc.vector.tensor_tensor(msq, mean, mean, op=OP.mult)
        ve = bb[0:1, 7:8]
        nc.vector.tensor_tensor(ve, e2, msq, op=OP.subtract)
        inv_ve = bb[0:1, 8:9]
        nc.vector.reciprocal(inv_ve, ve)
        rs = bb[0:1, 9:10]
        nc.scalar.activation(rs, inv_ve, AF.Sqrt)  # rs0 ~= 1/sqrt(ve) (table)
        # two Newton iterations: rs <- rs*(1.5 - 0.5*ve*rs^2)
        for it in range(2):
            r2 = bb[0:1, 10 + 3 * it : 11 + 3 * it]
            nc.vector.tensor_tensor(r2, rs, rs, op=OP.mult)
            pv = bb[0:1, 11 + 3 * it : 12 + 3 * it]
            nc.vector.tensor_tensor(pv, r2, ve, op=OP.mult)
            hh = bb[0:1, 12 + 3 * it : 13 + 3 * it]
            nc.vector.tensor_scalar(hh, pv, -0.5, 1.5, op0=OP.mult, op1=OP.add)
            rs_new = bb[0:1, 0:1] if it == 1 else bb[0:1, 16:17]
            nc.vector.tensor_tensor(rs_new, rs, hh, op=OP.mult)
            rs = rs_new
        # bb col0 = rs (final); col1 = -mean*rs
        mean_rs = bb[0:1, 17:18]
        nc.vector.tensor_tensor(mean_rs, mean, rs, op=OP.mult)
        nc.vector.tensor_scalar(bb[0:1, 1:2], mean_rs, -1.0, None, op0=OP.mult)
        bcast = bc_pool.tile([P, 2], f32)
        nc.gpsimd.partition_broadcast(bcast[:, 0:2], bb[0:1, 0:2])
        st_["bb"] = bb
        st_["ps_sm"] = ps_sm
        st_["rs_f"] = rs
        st_["mean"] = mean
        st_["rs_vec"] = bcast[:, 0:1]
        st_["nmrs_vec"] = bcast[:, 1:2]

    def loss(b):
        st_ = state[b]
        t_sb, s_sb = t_tiles[b], s_tiles[b]
        sums = st_["sums"]
        for ci in range(NCH):
            sl = slice(ci * CH, (ci + 1) * CH)
            tn = tn_pool.tile([P, CH], bf16)
            nc.scalar.activation(
                tn[:],
                t_sb[:, sl],
                AF.Identity,
                bias=st_["nmrs_vec"],
                scale=st_["rs_vec"],
            )
            dead = dead_pool.tile([P, CH], bf16, name="dead_kd")
            nc.vector._custom_dve(
                KD_OP,
                out=dead[:],
                in0=tn[:],
                in1=s_sb[:, sl],
                imm2=BETA,
                accum_out=sums[:, 2 + ci : 3 + ci],
            )
        ps_sm = st_["ps_sm"]
        nc.tensor.matmul(
            ps_sm[:, 2:4], ones_f32[:, :], sums[:, 2:4], start=True, stop=True
        )
        o = 8 * b
        nc.vector.reduce_sum(
            out=staging[0:1, o : o + 1], in_=ps_sm[0:1, 2:4], axis=AX.X
        )
        nc.vector.tensor_copy(staging[0:1, o + 1 : o + 2], st_["rs_f"])
        nc.vector.tensor_copy(staging[0:1, o + 2 : o + 3], st_["mean"])

    # software pipeline: stats lead loss by one sample
    stats(0)
    tiny(0)
    for b in range(BPC):
        if b + 1 < BPC:
            stats(b + 1)
        loss(b)
        if b + 1 < BPC:
            tiny(b + 1)

    nc.sync.dma_start(out_ap[:, :], staging[:, :])


_CACHED = {}


def _get_nc():
    if "nc" in _CACHED:
        return _CACHED["nc"]
    nc = bacc.Bacc(
        "TRN2",
        target_bir_lowering=False,
        debug=False,
        enable_asserts=False,
        num_devices=N_CORES,
    )
    teacher = nc.dram_tensor("teacher", [BPC, P, FD], f32, kind="ExternalInput").ap()
    stu = nc.dram_tensor("stu", [BPC, P, FD], f32, kind="ExternalInput").ap()
    out = nc.dram_tensor("out", [1, 8 * BPC], f32, kind="ExternalOutput").ap()
    with tile.TileContext(nc) as tc:
        with ExitStack() as ctx:
            _build_kernel(ctx, tc, out, teacher, stu)
    nc.compile()
    _CACHED["nc"] = nc
    return nc


def _combine(parts):
    """parts: list of 8 arrays [1, 8*BPC] -> scalar loss."""
    losses = []
    for r in parts:
        r = np.asarray(r, dtype=np.float64).reshape(BPC, 8)
        losses.append(0.25 * r[:, 0])
    losses = np.concatenate(losses)
    return np.float32(LOSS_WEIGHT * losses.mean())


def run(inputs: dict, trace: bool = False):
    teacher = np.ascontiguousarray(np.asarray(inputs["teacher_feat"], dtype=np.float32))
    stu = np.ascontiguousarray(np.asarray(inputs["stu_feat"], dtype=np.float32))
    assert teacher.shape == (B, C, H, W) and stu.shape == (B, C, H, W)
    tch = teacher.reshape(N_CORES, BPC, P, FD)
    sch = stu.reshape(N_CORES, BPC, P, FD)
    in_maps = [
        {"teacher": np.ascontiguousarray(tch[i]), "stu": np.ascontiguousarray(sch[i])}
        for i in range(N_CORES)
    ]
    nc = _get_nc()
    res = run_bass_kernel_spmd(nc, in_maps, core_ids=list(range(N_CORES)), trace=trace)
    parts = [res.results[i]["out"] for i in range(N_CORES)]
    return _combine(parts), res


def kernel(**inputs) -> np.ndarray:
    out, _ = run(inputs, trace=False)
    return np.asarray(out, dtype=np.float32)


if __name__ == "__main__":
    rng = np.random.default_rng(0)
    ins = {
        "teacher_feat": rng.standard_normal((B, C, H, W), dtype=np.float32),
        "stu_feat": rng.standard_normal((B, C, H, W), dtype=np.float32),
    }
    print(kernel(**ins))
